# revision 9
# baseline (speedup 1.0000x reference)
"""Trainium2 Bass kernel for the equivariant structure-denoising module.

Computation per node n:
    vec        = x[n, 1:4]                      # [3, 128]
    vec_local  = einsum('cf,ck->fk', vec, R)    # [128, 3]
    vec_norm   = sqrt(sum_c vec^2 + 1e-4)       # [128]
    a          = concat([x[n,0], vec_local.flat, vec_norm, extra[n]])   # [1152]
    h          = gelu(a @ W1 + b1)              # [1024], exact erf gelu
    y          = (h @ W2 + b2).reshape(4, 128)
    out        = concat([y[0:1], R @ y[1:4]])   # [4, 128]

Strategy (8 NeuronCores, data-parallel over nodes), bf16 end to end:
  - exact shard: 12500 nodes/core (no padding); 24 blocks of 512 nodes
    + 1 tail block of 212 (all compute trimmed to real nodes)
  - all tensor data in bf16 (error gate is 2e-2; bf16 keeps us ~5e-3)
  - activations feature-major A^T [1152, blk] per block:
      * x0/extra chunks pre-transposed on host -> straight DMA into A^T
      * rotated vec chunks computed feature-major directly on DVE: the
        host ships R entries broadcast across partitions (rb tile
        [128, 9*512] per block), so chunk k = sum_c xvt_c * rb[c,k] is
        15 large [128,512] tensor_tensor ops -- NO transposes anywhere
        (the baseline burned ~17us of PE on 300 transposes)
      * norm chunk: squares+adds on GpSimd (keeps ACT's activation
        table free of Square), one batched sqrt per block on ACT
        (bias=eps) directly into A^T[4]
  - MLP1: lhsT = W1 tile (stationary), rhs = A^T chunk -> PSUM H^T chunk
    (4 PSUM banks), GELU+bias on ScalarE PSUM->SBUF (bf16 out)
  - MLP2: lhsT = H^T chunk, rhs = W2 tile -> PSUM Y node-major across
    four banks; bias-add on DVE right after each subtile's matmuls,
    output rotation (node-major, per-partition R scalars from rs)
    deferred one subtile; output DMAs issue from the GpSimd queue to
    keep the Sync queue on input traffic only
  - host converts inputs fp32->bf16 and output bf16->fp32 (layout only,
    no math beyond dtype rounding + broadcast)
"""

import os
import sys

for _p in ("/opt/trn_rl_repo",):
    if _p not in sys.path and os.path.isdir(_p):
        sys.path.append(_p)

import ml_dtypes
import numpy as np

import concourse.bacc as bacc
import concourse.mybir as mybir
import concourse.tile as tile
from concourse.bass_utils import run_bass_kernel_spmd

F32 = mybir.dt.float32
BF16 = mybir.dt.bfloat16
NP_BF16 = np.dtype(ml_dtypes.bfloat16)

N_FULL = 100_000
N_CORES = 8
FIBER = 128
EXTRA = 512
HIDDEN = 1024
IN_DIM = FIBER * 5 + EXTRA   # 1152
OUT_DIM = FIBER * 4          # 512
EPS = 1e-4

BLK = 512                    # nodes per full block (PSUM bank = 512 fp32)
P = 128
NSUB = BLK // P              # 4 subtiles of 128 nodes
N_SHARD = N_FULL // N_CORES  # 12500 nodes per core
K_CH = IN_DIM // P           # 9 input chunks
H_CH = HIDDEN // P           # 8 hidden chunks
RB_W = 9 * BLK               # rb row width per block

# k accumulation order inside each MLP1 psum: chunks that are plain DMAs
# first (x0, extra), then the on-chip-computed ones (vec_local, norm) so
# block 0 can start before the rotation pipeline has produced anything.
K_ORDER = [0, 5, 6, 7, 8, 1, 2, 3, 4]


def shard_geometry(nshard):
    """(nblk, last_n): full 512-node blocks plus one partial tail block."""
    nblk = (nshard + BLK - 1) // BLK
    last_n = nshard - (nblk - 1) * BLK
    return nblk, last_n


def build_nc(nshard=N_SHARD):
    """Emit the per-core Bass program for nshard nodes."""
    nblk, last_n = shard_geometry(nshard)
    nc = bacc.Bacc(None, target_bir_lowering=False)

    xvt = nc.dram_tensor("xvt", [3 * FIBER, nshard], BF16, kind="ExternalInput")
    x0t = nc.dram_tensor("x0t", [P, nshard], BF16, kind="ExternalInput")
    et = nc.dram_tensor("et", [EXTRA, nshard], BF16, kind="ExternalInput")
    rb = nc.dram_tensor("rb", [P, nblk * RB_W], BF16, kind="ExternalInput")
    rs = nc.dram_tensor("rs", [nblk * P, NSUB * 16], F32, kind="ExternalInput")
    w1 = nc.dram_tensor("w1", [IN_DIM, HIDDEN], BF16, kind="ExternalInput")
    w2 = nc.dram_tensor("w2", [HIDDEN, OUT_DIM], BF16, kind="ExternalInput")
    b1r = nc.dram_tensor("b1r", [P, H_CH], F32, kind="ExternalInput")
    b2b = nc.dram_tensor("b2b", [P, OUT_DIM], F32, kind="ExternalInput")
    outs = nc.dram_tensor("out", [nshard, 4 * FIBER], BF16, kind="ExternalOutput")

    mult = mybir.AluOpType.mult
    add = mybir.AluOpType.add
    Sqrt = mybir.ActivationFunctionType.Sqrt
    Gelu = mybir.ActivationFunctionType.Gelu

    def blk_nodes(b):
        return BLK if b < nblk - 1 else last_n

    def sub_sizes(b):
        """List of subtile sizes (<=128 each) covering blk_nodes(b)."""
        nb = blk_nodes(b)
        sizes = []
        while nb > 0:
            sizes.append(min(P, nb))
            nb -= P
        return sizes

    def cols(t, nb):
        """Whole-tile AP for full blocks (keeps the PE fast path); slice
        only the partial tail block."""
        return t if nb == BLK else t[:, 0:nb]

    with tile.TileContext(nc) as tc:
        with (
            tc.tile_pool(name="consts", bufs=1) as consts,
            tc.tile_pool(name="at", bufs=2) as at_pool,
            tc.tile_pool(name="hsb", bufs=2) as h_pool,
            tc.tile_pool(name="vtt", bufs=2) as vt_pool,
            tc.tile_pool(name="rbp", bufs=2) as rb_pool,
            tc.tile_pool(name="sqp", bufs=2) as sq_pool,
            tc.tile_pool(name="rin", bufs=3) as r_pool,
            tc.tile_pool(name="vtmp", bufs=6) as v_pool,
            tc.tile_pool(name="ysb", bufs=4) as y_pool,
            tc.tile_pool(name="osb", bufs=4) as o_pool,
            tc.tile_pool(name="hpp", bufs=4, space="PSUM") as h_psum,
            tc.tile_pool(name="ypp", bufs=4, space="PSUM") as y_psum,
        ):
            eps_sb = consts.tile([P, 1], F32)
            nc.vector.memset(eps_sb, EPS)

            # W1 chunk DMAs split over two queues, K_ORDER-first so block 0
            # can begin accumulating as chunks land.
            w1_sb = [None] * K_CH
            for k in range(K_CH):
                w1_sb[k] = consts.tile([P, HIDDEN], BF16, name=f"w1_{k}",
                                       tag=f"w1_{k}")
            for qi, k in enumerate(K_ORDER):
                eng = nc.scalar if qi % 2 == 0 else nc.gpsimd
                eng.dma_start(out=w1_sb[k], in_=w1[k * P:(k + 1) * P, :])
            w2_sb = []
            for j in range(H_CH):
                t = consts.tile([P, OUT_DIM], BF16, name=f"w2_{j}", tag=f"w2_{j}")
                w2_sb.append(t)
            b1_sb = consts.tile([P, H_CH], F32)
            nc.gpsimd.dma_start(out=b1_sb, in_=b1r[:, :])
            b2_sb = consts.tile([P, OUT_DIM], F32)

            at_blocks = {}
            r_blocks = {}
            vt_blocks = {}
            rb_blocks = {}

            def emit_prep_a(b):
                n0 = b * BLK
                nb = blk_nodes(b)
                at = [at_pool.tile([P, BLK], BF16, name=f"at_{k}", tag=f"at_{k}")
                      for k in range(K_CH)]
                at_blocks[b] = at
                r_sb = r_pool.tile([P, NSUB * 16], F32, name="r_sb", tag="r_sb")
                nc.sync.dma_start(out=r_sb, in_=rs[b * P:(b + 1) * P, :])
                r_blocks[b] = r_sb

                # R broadcast rows + feature-major vec: they gate the
                # longest path (the rotation chunks)
                rb_sb = rb_pool.tile([P, RB_W], BF16, name="rb_sb", tag="rb_sb")
                nc.sync.dma_start(out=rb_sb, in_=rb[:, b * RB_W:b * RB_W + RB_W])
                rb_blocks[b] = rb_sb
                vt = vt_pool.tile([P, 3 * BLK], BF16, name="vt", tag="vt")
                for c in range(3):
                    nc.sync.dma_start(
                        out=vt[:, c * BLK:c * BLK + nb],
                        in_=xvt[c * P:(c + 1) * P, n0:n0 + nb])
                vt_blocks[b] = vt

                # x0 and extra chunks: straight DMA from host-transposed DRAM
                nc.sync.dma_start(out=cols(at[0], nb), in_=x0t[:, n0:n0 + nb])
                for t in range(4):
                    nc.sync.dma_start(
                        out=cols(at[5 + t], nb), in_=et[t * P:(t + 1) * P, n0:n0 + nb])

                # input rotations feature-major, straight into A^T chunks:
                # at[1+k][f, n] = sum_c vt_c[f, n] * R_n[c, k]
                def rbs(c, k):
                    s = (c * 3 + k) * BLK
                    return rb_sb[:, s:s + nb]

                for k in range(3):
                    m0 = v_pool.tile([P, BLK], BF16, name="rot_m0", tag="rot_m0")
                    m1 = v_pool.tile([P, BLK], BF16, name="rot_m1", tag="rot_m1")
                    m2 = v_pool.tile([P, BLK], BF16, name="rot_m2", tag="rot_m2")
                    nc.vector.tensor_mul(cols(m0, nb), vt[:, 0:nb], rbs(0, k))
                    nc.vector.tensor_mul(cols(m1, nb), vt[:, BLK:BLK + nb], rbs(1, k))
                    nc.vector.tensor_mul(cols(m2, nb), vt[:, 2 * BLK:2 * BLK + nb],
                                         rbs(2, k))
                    s = v_pool.tile([P, BLK], BF16, name="rot_s", tag="rot_s")
                    nc.vector.tensor_add(cols(s, nb), cols(m0, nb), cols(m1, nb))
                    nc.vector.tensor_add(cols(at[1 + k], nb), cols(s, nb),
                                         cols(m2, nb))

            def emit_prep_b(b):
                at = at_blocks[b]
                nb = blk_nodes(b)
                # norm chunk: squares+adds on GpSimd (no ACT table), one
                # sqrt per block on ACT (bias=eps) directly into A^T[4]
                vt = vt_blocks.pop(b)
                sqs = []
                for c in range(3):
                    s = sq_pool.tile([P, BLK], BF16, name=f"s_{c}", tag=f"s_{c}")
                    nc.gpsimd.tensor_mul(cols(s, nb), vt[:, c * BLK:c * BLK + nb],
                                         vt[:, c * BLK:c * BLK + nb])
                    sqs.append(s)
                t01 = sq_pool.tile([P, BLK], BF16, name="t01", tag="t01")
                nc.gpsimd.tensor_add(cols(t01, nb), cols(sqs[0], nb), cols(sqs[1], nb))
                nsq = sq_pool.tile([P, BLK], BF16, name="nsq", tag="nsq")
                nc.gpsimd.tensor_add(cols(nsq, nb), cols(t01, nb), cols(sqs[2], nb))
                nc.scalar.activation(cols(at[4], nb), cols(nsq, nb), Sqrt, bias=eps_sb)

            h_blocks = {}

            def emit_mlp1(b):
                at = at_blocks.pop(b)
                nb = blk_nodes(b)
                h_sb = []
                for j in range(H_CH):
                    hp = h_psum.tile([P, BLK], F32, name="hp", tag="hp")
                    for ki, k in enumerate(K_ORDER):
                        nc.tensor.matmul(
                            cols(hp, nb),
                            w1_sb[k][:, j * P:(j + 1) * P],
                            cols(at[k], nb),
                            start=(ki == 0), stop=(ki == K_CH - 1))
                    h = h_pool.tile([P, BLK], BF16, name=f"h_{j}", tag=f"h_{j}")
                    nc.scalar.activation(cols(h, nb), cols(hp, nb), Gelu,
                                         bias=b1_sb[:, j:j + 1])
                    h_sb.append(h)
                h_blocks[b] = h_sb

            def emit_mlp2(b):
                n0 = b * BLK
                subs = sub_sizes(b)
                r_sb = r_blocks.pop(b)
                rb_blocks.pop(b)
                h_sb = h_blocks.pop(b)

                def emit_rot_out(i, y_sb):
                    base = n0 + i * P
                    sn = subs[i]

                    def rsc(c, k):
                        col = i * 16 + c * 3 + k
                        rcol = r_sb if sn == P else r_sb[0:sn, :]
                        return rcol[:, col:col + 1]

                    # scalar part goes straight out
                    nc.gpsimd.dma_start(
                        out=outs[base:base + sn, 0:P], in_=y_sb[:, 0:P])
                    o_sb = o_pool.tile([sn, 3 * P], BF16, name="o_sb", tag="o_sb")
                    yv0 = y_sb[:, 1 * P:2 * P]
                    yv1 = y_sb[:, 2 * P:3 * P]
                    yv2 = y_sb[:, 3 * P:4 * P]
                    # vec_out_c = R[c,0]*yv0 + R[c,1]*yv1 + R[c,2]*yv2 (DVE)
                    for c in range(3):
                        ta = v_pool.tile([sn, P], BF16, name="orot_a", tag="orot_a")
                        tb = v_pool.tile([sn, P], BF16, name="orot_b", tag="orot_b")
                        if b == nblk - 1:
                            nc.scalar.mul(ta, yv0, rsc(c, 0))
                        else:
                            nc.vector.tensor_scalar_mul(ta, yv0, rsc(c, 0))
                        nc.vector.scalar_tensor_tensor(
                            tb, yv1, rsc(c, 1), ta, op0=mult, op1=add)
                        nc.vector.scalar_tensor_tensor(
                            o_sb[:, c * P:(c + 1) * P], yv2, rsc(c, 2),
                            tb, op0=mult, op1=add)
                    nc.gpsimd.dma_start(
                        out=outs[base:base + sn, P:4 * P], in_=o_sb)

                # bias-add goes on DVE right after each subtile's matmuls so
                # the PSUM bank frees in time; the 9-op rotation of subtile
                # i is deferred until after bias(i+1) is enqueued
                pend = []
                for i, sn in enumerate(subs):
                    yp = y_psum.tile([sn, OUT_DIM], F32, name="yp", tag="yp")
                    for j in range(H_CH):
                        nc.tensor.matmul(
                            yp,
                            h_sb[j][:, i * P:i * P + sn],
                            w2_sb[j],
                            start=(j == 0), stop=(j == H_CH - 1))
                    y_sb = y_pool.tile([sn, OUT_DIM], BF16, name="y_sb", tag="y_sb")
                    nc.vector.tensor_add(y_sb, yp,
                                         b2_sb if sn == P else b2_sb[0:sn, :])
                    if pend:
                        emit_rot_out(*pend.pop())
                    pend.append((i, y_sb))
                emit_rot_out(*pend.pop())

            # software pipeline; emission order = Tile priority / queue order
            emit_prep_a(0)
            emit_prep_b(0)
            # W2/b2 first needed by MLP2 of block 0 (~25us in): defer their
            # DMAs behind block-0 input prep
            for j in range(H_CH):
                nc.gpsimd.dma_start(out=w2_sb[j], in_=w2[j * P:(j + 1) * P, :])
            nc.gpsimd.dma_start(out=b2_sb, in_=b2b[:, :])
            for b in range(nblk):
                if b + 1 < nblk:
                    emit_prep_a(b + 1)
                emit_mlp1(b)
                if b + 1 < nblk:
                    emit_prep_b(b + 1)
                emit_mlp2(b)

    nc.finalize()
    return nc


def prep_inputs(x, rotation_mats, extra_feats, W1, b1, W2, b2, nshard=N_SHARD):
    """Host-side shard + layout massaging. Returns per-core input maps."""
    nblk, last_n = shard_geometry(nshard)
    n = x.shape[0]
    assert n == nshard * N_CORES, (n, nshard)

    x = np.asarray(x, dtype=np.float32)
    xv = np.ascontiguousarray(x[:, 1:4, :]).reshape(n, 3 * FIBER).astype(NP_BF16)
    x0 = np.ascontiguousarray(x[:, 0, :]).astype(NP_BF16)            # [n, 128]
    r = np.ascontiguousarray(np.asarray(rotation_mats, dtype=np.float32).reshape(n, 9))
    e = np.asarray(extra_feats, dtype=np.float32).astype(NP_BF16)

    # W1 rows permuted: our A^T row order is [x0; vl_k k-major; vn; extra],
    # reference is [x0; vl (f,k) f-major; vn; extra]
    perm = np.arange(IN_DIM)
    for k in range(3):
        perm[P + k * P + np.arange(P)] = P + np.arange(P) * 3 + k
    w1p = np.ascontiguousarray(np.asarray(W1, dtype=np.float32)[perm, :]).astype(NP_BF16)
    w2 = np.ascontiguousarray(np.asarray(W2, dtype=np.float32)).astype(NP_BF16)
    b1r = np.ascontiguousarray(np.asarray(b1, dtype=np.float32).reshape(H_CH, P).T)
    b2b = np.ascontiguousarray(np.tile(np.asarray(b2, dtype=np.float32), (P, 1)))

    npad = nblk * BLK   # per-core node count padded to whole blocks
    in_maps = []
    for c in range(N_CORES):
        sl = slice(c * nshard, (c + 1) * nshard)
        # R scalars node-major for the output rotation (zero-padded)
        rc_full = np.zeros((npad, 9), np.float32)
        rc_full[:nshard] = r[sl]
        rc = rc_full.reshape(nblk, NSUB, P, 9).transpose(0, 2, 1, 3)  # [nblk,P,NSUB,9]
        rc16 = np.zeros((nblk, P, NSUB, 16), np.float32)
        rc16[..., :9] = rc
        # R rows broadcast across partitions for the input rotation:
        # rb[p, b*RB_W + (c*3+k)*BLK + j] = R[b*BLK + j][c, k]
        rb_rows = np.zeros((nblk, 9, BLK), NP_BF16)
        rb_rows[..., :] = 0
        src = rc_full.astype(NP_BF16).reshape(nblk, BLK, 9).transpose(0, 2, 1)
        rb_rows[:, :, :] = src
        rb_flat = rb_rows.reshape(1, nblk * RB_W)
        rb_bcast = np.ascontiguousarray(np.broadcast_to(rb_flat, (P, nblk * RB_W)))
        in_maps.append({
            "xvt": np.ascontiguousarray(xv[sl].reshape(nshard, 3, FIBER)
                                        .transpose(1, 2, 0).reshape(3 * FIBER, nshard)),
            "x0t": np.ascontiguousarray(x0[sl].T),
            "et": np.ascontiguousarray(e[sl].T),
            "rb": rb_bcast,
            "rs": rc16.reshape(nblk * P, NSUB * 16),
            "w1": w1p,
            "w2": w2,
            "b1r": b1r,
            "b2b": b2b,
        })
    return in_maps


_NC_CACHE = {}


def run(x, rotation_mats, extra_feats, W1, b1, W2, b2,
        nshard=None, trace=False, use_f32r=True):
    n = x.shape[0]
    if nshard is None:
        nshard = n // N_CORES
    key = (nshard,)
    if key not in _NC_CACHE:
        _NC_CACHE[key] = build_nc(nshard=nshard)
    nc = _NC_CACHE[key]
    in_maps = prep_inputs(x, rotation_mats, extra_feats, W1, b1, W2, b2,
                          nshard=nshard)
    res = run_bass_kernel_spmd(nc, in_maps, list(range(N_CORES)), trace=trace)
    full = np.concatenate([res.results[c]["out"] for c in range(N_CORES)], axis=0)
    out = full[:n].astype(np.float32).reshape(n, 4, FIBER)
    return out, res


def kernel(x, rotation_mats, extra_feats, W1, b1, W2, b2):
    out, _ = run(x, rotation_mats, extra_feats, W1, b1, W2, b2)
    return out


# revision 11
# speedup vs baseline: 1.1865x; 1.1865x over previous
"""Trainium2 Bass kernel for the equivariant structure-denoising module.

Computation per node n:
    vec        = x[n, 1:4]                      # [3, 128]
    vec_local  = einsum('cf,ck->fk', vec, R)    # [128, 3]
    vec_norm   = sqrt(sum_c vec^2 + 1e-4)       # [128]
    a          = concat([x[n,0], vec_local.flat, vec_norm, extra[n]])   # [1152]
    h          = gelu(a @ W1 + b1)              # [1024], exact erf gelu
    y          = (h @ W2 + b2).reshape(4, 128)
    out        = concat([y[0:1], R @ y[1:4]])   # [4, 128]

Strategy (8 NeuronCores, data-parallel over nodes), bf16 end to end:
  - exact shard: 12500 nodes/core (no padding); 24 blocks of 512 nodes
    + 1 tail block of 212 (all compute trimmed to real nodes)
  - all tensor data in bf16 (error gate is 2e-2; bf16 keeps us ~5e-3)
  - activations feature-major A^T [1152, blk] per block:
      * x0/extra chunks pre-transposed on host -> straight DMA into A^T
      * rotated vec chunks computed feature-major directly on DVE: the
        host ships R entries broadcast across partitions (rb tile
        [128, 9*512] per block), so chunk k = sum_c xvt_c * rb[c,k] is
        15 large [128,512] tensor_tensor ops -- NO transposes anywhere
        (the baseline burned ~17us of PE on 300 transposes)
      * norm chunk: squares+adds on GpSimd (keeps ACT's activation
        table free of Square), one batched sqrt per block on ACT
        (bias=eps) directly into A^T[4]
  - MLP1: lhsT = W1 tile (stationary), rhs = A^T chunk -> PSUM H^T chunk
    (4 PSUM banks), GELU+bias on ScalarE PSUM->SBUF (bf16 out)
  - MLP2: lhsT = H^T chunk, rhs = W2 tile -> PSUM Y node-major across
    four banks; bias-add on DVE right after each subtile's matmuls,
    output rotation (node-major, per-partition R scalars from rs)
    deferred one subtile; output DMAs issue from the GpSimd queue to
    keep the Sync queue on input traffic only
  - host converts inputs fp32->bf16 and output bf16->fp32 (layout only,
    no math beyond dtype rounding + broadcast)
"""

import os
import sys

for _p in ("/opt/trn_rl_repo",):
    if _p not in sys.path and os.path.isdir(_p):
        sys.path.append(_p)

import ml_dtypes
import numpy as np

import concourse.bacc as bacc
import concourse.mybir as mybir
import concourse.tile as tile
from concourse.bass_utils import run_bass_kernel_spmd

F32 = mybir.dt.float32
BF16 = mybir.dt.bfloat16
NP_BF16 = np.dtype(ml_dtypes.bfloat16)

N_FULL = 100_000
N_CORES = 8
FIBER = 128
EXTRA = 512
HIDDEN = 1024
IN_DIM = FIBER * 5 + EXTRA   # 1152
OUT_DIM = FIBER * 4          # 512
EPS = 1e-4

BLK = 512                    # nodes per full block (PSUM bank = 512 fp32)
P = 128
NSUB = BLK // P              # 4 subtiles of 128 nodes
N_SHARD = N_FULL // N_CORES  # 12500 nodes per core
K_CH = IN_DIM // P           # 9 input chunks
H_CH = HIDDEN // P           # 8 hidden chunks
RB_W = 9 * BLK               # rb row width per block

# k accumulation order inside each MLP1 psum: chunks that are plain DMAs
# first (x0, extra), then the on-chip-computed ones (vec_local, norm) so
# block 0 can start before the rotation pipeline has produced anything.
K_ORDER = [0, 5, 6, 7, 8, 1, 2, 3, 4]


def shard_geometry(nshard):
    """(nblk, last_n): full 512-node blocks plus one partial tail block."""
    nblk = (nshard + BLK - 1) // BLK
    last_n = nshard - (nblk - 1) * BLK
    return nblk, last_n


def build_nc(nshard=N_SHARD):
    """Emit the per-core Bass program for nshard nodes."""
    nblk, last_n = shard_geometry(nshard)
    nc = bacc.Bacc(None, target_bir_lowering=False)

    xvt = nc.dram_tensor("xvt", [3 * FIBER, nshard], BF16, kind="ExternalInput")
    x0t = nc.dram_tensor("x0t", [P, nshard], BF16, kind="ExternalInput")
    et = nc.dram_tensor("et", [EXTRA, nshard], BF16, kind="ExternalInput")
    rb = nc.dram_tensor("rb", [P, nblk * RB_W], BF16, kind="ExternalInput")
    rs = nc.dram_tensor("rs", [nblk * P, NSUB * 16], F32, kind="ExternalInput")
    w1 = nc.dram_tensor("w1", [IN_DIM, HIDDEN], BF16, kind="ExternalInput")
    w2 = nc.dram_tensor("w2", [HIDDEN, OUT_DIM], BF16, kind="ExternalInput")
    b1r = nc.dram_tensor("b1r", [P, H_CH], F32, kind="ExternalInput")
    b2b = nc.dram_tensor("b2b", [P, OUT_DIM], F32, kind="ExternalInput")
    outs = nc.dram_tensor("out", [nshard, 4 * FIBER], BF16, kind="ExternalOutput")

    mult = mybir.AluOpType.mult
    add = mybir.AluOpType.add
    Sqrt = mybir.ActivationFunctionType.Sqrt
    Square = mybir.ActivationFunctionType.Square
    Gelu = mybir.ActivationFunctionType.Gelu

    def blk_nodes(b):
        return BLK if b < nblk - 1 else last_n

    def sub_sizes(b):
        """List of subtile sizes (<=128 each) covering blk_nodes(b)."""
        nb = blk_nodes(b)
        sizes = []
        while nb > 0:
            sizes.append(min(P, nb))
            nb -= P
        return sizes

    def cols(t, nb):
        """Whole-tile AP for full blocks (keeps the PE fast path); slice
        only the partial tail block."""
        return t if nb == BLK else t[:, 0:nb]

    with tile.TileContext(nc) as tc:
        with (
            tc.tile_pool(name="consts", bufs=1) as consts,
            tc.tile_pool(name="at", bufs=2) as at_pool,
            tc.tile_pool(name="hsb", bufs=2) as h_pool,
            tc.tile_pool(name="vtt", bufs=2) as vt_pool,
            tc.tile_pool(name="rbp", bufs=2) as rb_pool,
            tc.tile_pool(name="sqp", bufs=2) as sq_pool,
            tc.tile_pool(name="rin", bufs=3) as r_pool,
            tc.tile_pool(name="vtmp", bufs=6) as v_pool,
            tc.tile_pool(name="ysb", bufs=4) as y_pool,
            tc.tile_pool(name="osb", bufs=4) as o_pool,
            tc.tile_pool(name="hpp", bufs=4, space="PSUM") as h_psum,
            tc.tile_pool(name="ypp", bufs=4, space="PSUM") as y_psum,
        ):
            eps_sb = consts.tile([P, 1], F32)
            nc.vector.memset(eps_sb, EPS)

            # W1 chunk DMAs split over two queues, K_ORDER-first so block 0
            # can begin accumulating as chunks land.
            w1_sb = [None] * K_CH
            for k in range(K_CH):
                w1_sb[k] = consts.tile([P, HIDDEN], BF16, name=f"w1_{k}",
                                       tag=f"w1_{k}")
            for qi, k in enumerate(K_ORDER):
                eng = nc.scalar if qi % 2 == 0 else nc.gpsimd
                eng.dma_start(out=w1_sb[k], in_=w1[k * P:(k + 1) * P, :])
            w2_sb = []
            for j in range(H_CH):
                t = consts.tile([P, OUT_DIM], BF16, name=f"w2_{j}", tag=f"w2_{j}")
                w2_sb.append(t)
            b1_sb = consts.tile([P, H_CH], F32)
            nc.gpsimd.dma_start(out=b1_sb, in_=b1r[:, :])
            b2_sb = consts.tile([P, OUT_DIM], F32)

            at_blocks = {}
            r_blocks = {}
            vt_blocks = {}
            rb_blocks = {}

            def emit_prep_a(b):
                n0 = b * BLK
                nb = blk_nodes(b)
                at = [at_pool.tile([P, BLK], BF16, name=f"at_{k}", tag=f"at_{k}")
                      for k in range(K_CH)]
                at_blocks[b] = at
                r_sb = r_pool.tile([P, NSUB * 16], F32, name="r_sb", tag="r_sb")
                nc.sync.dma_start(out=r_sb, in_=rs[b * P:(b + 1) * P, :])
                r_blocks[b] = r_sb

                # R broadcast rows + feature-major vec: they gate the
                # longest path (the rotation chunks)
                rb_sb = rb_pool.tile([P, RB_W], BF16, name="rb_sb", tag="rb_sb")
                nc.sync.dma_start(out=rb_sb, in_=rb[:, b * RB_W:b * RB_W + RB_W])
                rb_blocks[b] = rb_sb
                vt = vt_pool.tile([P, 3 * BLK], BF16, name="vt", tag="vt")
                for c in range(3):
                    nc.sync.dma_start(
                        out=vt[:, c * BLK:c * BLK + nb],
                        in_=xvt[c * P:(c + 1) * P, n0:n0 + nb])
                vt_blocks[b] = vt

                # x0 and extra chunks: straight DMA from host-transposed DRAM
                nc.sync.dma_start(out=cols(at[0], nb), in_=x0t[:, n0:n0 + nb])
                for t in range(4):
                    nc.sync.dma_start(
                        out=cols(at[5 + t], nb), in_=et[t * P:(t + 1) * P, n0:n0 + nb])

                # input rotations feature-major, straight into A^T chunks:
                # at[1+k][f, n] = sum_c vt_c[f, n] * R_n[c, k]
                def rbs(c, k):
                    s = (c * 3 + k) * BLK
                    return rb_sb[:, s:s + nb]

                for k in range(3):
                    m0 = v_pool.tile([P, BLK], BF16, name="rot_m0", tag="rot_m0")
                    m1 = v_pool.tile([P, BLK], BF16, name="rot_m1", tag="rot_m1")
                    m2 = v_pool.tile([P, BLK], BF16, name="rot_m2", tag="rot_m2")
                    nc.vector.tensor_mul(cols(m0, nb), vt[:, 0:nb], rbs(0, k))
                    nc.vector.tensor_mul(cols(m1, nb), vt[:, BLK:BLK + nb], rbs(1, k))
                    nc.vector.tensor_mul(cols(m2, nb), vt[:, 2 * BLK:2 * BLK + nb],
                                         rbs(2, k))
                    s = v_pool.tile([P, BLK], BF16, name="rot_s", tag="rot_s")
                    nc.vector.tensor_add(cols(s, nb), cols(m0, nb), cols(m1, nb))
                    nc.vector.tensor_add(cols(at[1 + k], nb), cols(s, nb),
                                         cols(m2, nb))

            def emit_prep_b(b):
                at = at_blocks[b]
                nb = blk_nodes(b)
                # norm chunk: squares+adds on GpSimd (no ACT table), one
                # sqrt per block on ACT (bias=eps) directly into A^T[4]
                vt = vt_blocks.pop(b)
                sqs = []
                for c in range(3):
                    s = sq_pool.tile([P, BLK], BF16, name=f"s_{c}", tag=f"s_{c}")
                    nc.scalar.activation(cols(s, nb), vt[:, c * BLK:c * BLK + nb],
                                         Square)
                    sqs.append(s)
                t01 = sq_pool.tile([P, BLK], BF16, name="t01", tag="t01")
                nc.gpsimd.tensor_add(cols(t01, nb), cols(sqs[0], nb), cols(sqs[1], nb))
                nsq = sq_pool.tile([P, BLK], BF16, name="nsq", tag="nsq")
                nc.gpsimd.tensor_add(cols(nsq, nb), cols(t01, nb), cols(sqs[2], nb))
                nc.scalar.activation(cols(at[4], nb), cols(nsq, nb), Sqrt, bias=eps_sb)

            h_blocks = {}

            def emit_mlp1(b):
                at = at_blocks.pop(b)
                nb = blk_nodes(b)
                h_sb = []
                for j in range(H_CH):
                    hp = h_psum.tile([P, BLK], F32, name="hp", tag="hp")
                    for ki, k in enumerate(K_ORDER):
                        nc.tensor.matmul(
                            cols(hp, nb),
                            w1_sb[k][:, j * P:(j + 1) * P],
                            cols(at[k], nb),
                            start=(ki == 0), stop=(ki == K_CH - 1))
                    h = h_pool.tile([P, BLK], BF16, name=f"h_{j}", tag=f"h_{j}")
                    nc.scalar.activation(cols(h, nb), cols(hp, nb), Gelu,
                                         bias=b1_sb[:, j:j + 1])
                    h_sb.append(h)
                h_blocks[b] = h_sb

            def emit_mlp2(b):
                n0 = b * BLK
                subs = sub_sizes(b)
                r_sb = r_blocks.pop(b)
                rb_blocks.pop(b)
                h_sb = h_blocks.pop(b)

                def emit_rot_out(i, y_sb):
                    base = n0 + i * P
                    sn = subs[i]

                    def rsc(c, k):
                        col = i * 16 + c * 3 + k
                        rcol = r_sb if sn == P else r_sb[0:sn, :]
                        return rcol[:, col:col + 1]

                    # scalar part goes straight out
                    nc.gpsimd.dma_start(
                        out=outs[base:base + sn, 0:P], in_=y_sb[:, 0:P])
                    o_sb = o_pool.tile([sn, 3 * P], BF16, name="o_sb", tag="o_sb")
                    yv0 = y_sb[:, 1 * P:2 * P]
                    yv1 = y_sb[:, 2 * P:3 * P]
                    yv2 = y_sb[:, 3 * P:4 * P]
                    # vec_out_c = R[c,0]*yv0 + R[c,1]*yv1 + R[c,2]*yv2 (DVE)
                    for c in range(3):
                        ta = v_pool.tile([sn, P], BF16, name="orot_a", tag="orot_a")
                        tb = v_pool.tile([sn, P], BF16, name="orot_b", tag="orot_b")
                        if b == nblk - 1:
                            nc.scalar.mul(ta, yv0, rsc(c, 0))
                        else:
                            nc.vector.tensor_scalar_mul(ta, yv0, rsc(c, 0))
                        nc.vector.scalar_tensor_tensor(
                            tb, yv1, rsc(c, 1), ta, op0=mult, op1=add)
                        nc.vector.scalar_tensor_tensor(
                            o_sb[:, c * P:(c + 1) * P], yv2, rsc(c, 2),
                            tb, op0=mult, op1=add)
                    nc.gpsimd.dma_start(
                        out=outs[base:base + sn, P:4 * P], in_=o_sb)

                # bias-add goes on DVE right after each subtile's matmuls so
                # the PSUM bank frees in time; the 9-op rotation of subtile
                # i is deferred until after bias(i+1) is enqueued
                pend = []
                for i, sn in enumerate(subs):
                    yp = y_psum.tile([sn, OUT_DIM], F32, name="yp", tag="yp")
                    for j in range(H_CH):
                        nc.tensor.matmul(
                            yp,
                            h_sb[j][:, i * P:i * P + sn],
                            w2_sb[j],
                            start=(j == 0), stop=(j == H_CH - 1))
                    y_sb = y_pool.tile([sn, OUT_DIM], BF16, name="y_sb", tag="y_sb")
                    nc.vector.tensor_add(y_sb, yp,
                                         b2_sb if sn == P else b2_sb[0:sn, :])
                    if pend:
                        emit_rot_out(*pend.pop())
                    pend.append((i, y_sb))
                emit_rot_out(*pend.pop())

            # software pipeline; emission order = Tile priority / queue order
            emit_prep_a(0)
            emit_prep_b(0)
            # W2/b2 first needed by MLP2 of block 0 (~25us in): defer their
            # DMAs behind block-0 input prep
            for j in range(H_CH):
                nc.gpsimd.dma_start(out=w2_sb[j], in_=w2[j * P:(j + 1) * P, :])
            nc.gpsimd.dma_start(out=b2_sb, in_=b2b[:, :])
            for b in range(nblk):
                if b + 1 < nblk:
                    emit_prep_a(b + 1)
                emit_mlp1(b)
                if b + 1 < nblk:
                    emit_prep_b(b + 1)
                emit_mlp2(b)

    nc.finalize()
    return nc


def prep_inputs(x, rotation_mats, extra_feats, W1, b1, W2, b2, nshard=N_SHARD):
    """Host-side shard + layout massaging. Returns per-core input maps."""
    nblk, last_n = shard_geometry(nshard)
    n = x.shape[0]
    assert n == nshard * N_CORES, (n, nshard)

    x = np.asarray(x, dtype=np.float32)
    xv = np.ascontiguousarray(x[:, 1:4, :]).reshape(n, 3 * FIBER).astype(NP_BF16)
    x0 = np.ascontiguousarray(x[:, 0, :]).astype(NP_BF16)            # [n, 128]
    r = np.ascontiguousarray(np.asarray(rotation_mats, dtype=np.float32).reshape(n, 9))
    e = np.asarray(extra_feats, dtype=np.float32).astype(NP_BF16)

    # W1 rows permuted: our A^T row order is [x0; vl_k k-major; vn; extra],
    # reference is [x0; vl (f,k) f-major; vn; extra]
    perm = np.arange(IN_DIM)
    for k in range(3):
        perm[P + k * P + np.arange(P)] = P + np.arange(P) * 3 + k
    w1p = np.ascontiguousarray(np.asarray(W1, dtype=np.float32)[perm, :]).astype(NP_BF16)
    w2 = np.ascontiguousarray(np.asarray(W2, dtype=np.float32)).astype(NP_BF16)
    b1r = np.ascontiguousarray(np.asarray(b1, dtype=np.float32).reshape(H_CH, P).T)
    b2b = np.ascontiguousarray(np.tile(np.asarray(b2, dtype=np.float32), (P, 1)))

    npad = nblk * BLK   # per-core node count padded to whole blocks
    in_maps = []
    for c in range(N_CORES):
        sl = slice(c * nshard, (c + 1) * nshard)
        # R scalars node-major for the output rotation (zero-padded)
        rc_full = np.zeros((npad, 9), np.float32)
        rc_full[:nshard] = r[sl]
        rc = rc_full.reshape(nblk, NSUB, P, 9).transpose(0, 2, 1, 3)  # [nblk,P,NSUB,9]
        rc16 = np.zeros((nblk, P, NSUB, 16), np.float32)
        rc16[..., :9] = rc
        # R rows broadcast across partitions for the input rotation:
        # rb[p, b*RB_W + (c*3+k)*BLK + j] = R[b*BLK + j][c, k]
        rb_rows = np.zeros((nblk, 9, BLK), NP_BF16)
        rb_rows[..., :] = 0
        src = rc_full.astype(NP_BF16).reshape(nblk, BLK, 9).transpose(0, 2, 1)
        rb_rows[:, :, :] = src
        rb_flat = rb_rows.reshape(1, nblk * RB_W)
        rb_bcast = np.ascontiguousarray(np.broadcast_to(rb_flat, (P, nblk * RB_W)))
        in_maps.append({
            "xvt": np.ascontiguousarray(xv[sl].reshape(nshard, 3, FIBER)
                                        .transpose(1, 2, 0).reshape(3 * FIBER, nshard)),
            "x0t": np.ascontiguousarray(x0[sl].T),
            "et": np.ascontiguousarray(e[sl].T),
            "rb": rb_bcast,
            "rs": rc16.reshape(nblk * P, NSUB * 16),
            "w1": w1p,
            "w2": w2,
            "b1r": b1r,
            "b2b": b2b,
        })
    return in_maps


_NC_CACHE = {}


def run(x, rotation_mats, extra_feats, W1, b1, W2, b2,
        nshard=None, trace=False, use_f32r=True):
    n = x.shape[0]
    if nshard is None:
        nshard = n // N_CORES
    key = (nshard,)
    if key not in _NC_CACHE:
        _NC_CACHE[key] = build_nc(nshard=nshard)
    nc = _NC_CACHE[key]
    in_maps = prep_inputs(x, rotation_mats, extra_feats, W1, b1, W2, b2,
                          nshard=nshard)
    res = run_bass_kernel_spmd(nc, in_maps, list(range(N_CORES)), trace=trace)
    full = np.concatenate([res.results[c]["out"] for c in range(N_CORES)], axis=0)
    out = full[:n].astype(np.float32).reshape(n, 4, FIBER)
    return out, res


def kernel(x, rotation_mats, extra_feats, W1, b1, W2, b2):
    out, _ = run(x, rotation_mats, extra_feats, W1, b1, W2, b2)
    return out
